# revision 11
# baseline (speedup 1.0000x reference)
"""MixConv depthwise conv (3x3/5x5/7x7 over 64-channel groups) as banded-Toeplitz
matmuls on the TensorEngine, sharded over 8 NeuronCores by channel.

Decomposition: a kxk depthwise conv = sum over dx of a 1D conv along H applied to
the input shifted by dx along W. The 1D conv along H is a matmul with a banded
[H, H] Toeplitz matrix (built host-side from the conv weights) contracting over
H=112 partitions. W-shifts are free-dim offsets into a padded SBUF image tile;
the dx-passes accumulate in PSUM. Matmul rhs uses a segmented AP ([4 images x
112 cols], stride 118) so the inter-image pad columns are never streamed; PE
runs at the 1 col/cycle bf16 roofline with ~197 ns/matmul pitch.

Sharding: 192 channels / 8 cores = 24 channels per core, 8 from each kernel-size
group so PE work is balanced. Channels run k-descending (7,5,3) so DMA prefetch
builds headroom while the PE chews the big kernels.

All HBM traffic is bf16 (x, Toeplitz weights, y): fp32 staging makes the kernel
DMA-bound with the PE HAM-throttled. PSUM accumulation stays fp32; measured
~4e-3 max rel err vs the 2e-2 gate. Latency shaping: x loads ride the SP HWDGE
ring as channel-pair transfers (first pair split so ch0 starts early), y stores
and the per-channel Toeplitz tables ride the ACT ring, and a short burst of
dependency-free warmup matmuls on a memset scratch tile keeps the PE busy and
HAM-warm while the first transfers land.
"""

import numpy as np
import ml_dtypes

import concourse.bacc as bacc
import concourse.mybir as mybir
import concourse.tile as tile
from concourse.bass_utils import run_bass_kernel_spmd

# Problem constants (hardcoded per contract)
N_IMGS = 32
H = W = 112
GROUP_KS = (7, 5, 3)     # device processing order: k-descending
GROUP_SIZE = 64          # channels per group
N_CORES = 8
CH_PER_GROUP_PER_CORE = GROUP_SIZE // N_CORES   # 8
CH_PER_CORE = CH_PER_GROUP_PER_CORE * len(GROUP_KS)  # 24
N_PAIRS = CH_PER_CORE // 2

RW = W + 6               # per-image region width in the padded tile (max pad=3)
DATA_OFF = 3             # data cols at [3, 115) of each region
XCOLS = N_IMGS * RW + 6  # 3782 — +6 so the last bank's 4*RW slice stays in range
OCOLS = N_IMGS * W       # 3584
N_BANKS = 8              # image chunks / PSUM banks per channel
IMG_PER_BANK = N_IMGS // N_BANKS  # 4
N_MM = IMG_PER_BANK * W  # 448 — matmul free size (segmented, pads skipped)
PS_STRIDE = 512          # fp32 slots per PSUM bank; 2 banks per PSUM tile
N_WARMUP = 12            # dep-free matmuls to keep PE busy+warm during head DMAs
XSPLIT = RW * 16 + 8     # 1896 — ch0 half-load split (covers banks 0-3 reads)

KS = [7] * 8 + [5] * 8 + [3] * 8          # per-channel kernel size (device order)
TOFF = np.cumsum([0] + KS).tolist()       # tmat col-block offset per channel
N_TMAT = TOFF[-1]                          # 120 [H,H] Toeplitz slices
TCOLS = N_TMAT * H                         # 13440

MM_MODE = "bf16"

_BASS_CACHE = {}


def _build_bass():
    bf16 = mybir.dt.bfloat16
    f32 = mybir.dt.float32

    nc = bacc.Bacc("TRN2", target_bir_lowering=False, debug=False)
    xp_d = nc.dram_tensor(
        "xp", [N_PAIRS, H, 2 * XCOLS], bf16, kind="ExternalInput"
    )
    t_d = nc.dram_tensor("tmat", [H, TCOLS], bf16, kind="ExternalInput")
    y_d = nc.dram_tensor(
        "y", [CH_PER_CORE, H, OCOLS], bf16, kind="ExternalOutput"
    )

    with tile.TileContext(nc) as tc:
        with (
            tc.tile_pool(name="xpool", bufs=3) as xpool,
            tc.tile_pool(name="tpool", bufs=1) as tpool,
            tc.tile_pool(name="opool", bufs=3) as opool,
            tc.tile_pool(name="wpool", bufs=1) as wpool,
            tc.tile_pool(name="pspool", bufs=N_BANKS // 2, space="PSUM") as pspool,
        ):
            # PE warmup: dep-free matmuls on a memset tile fill the initial
            # DMA wait and take the HAM clock-gate to 8/8 before real work.
            w_t = wpool.tile([H, N_MM], bf16, tag="warm", name="warm")
            nc.any.memset(w_t[:, :], 0.0)
            pw = pspool.tile([H, 2 * PS_STRIDE], f32, tag="ps", name="ps_warm")
            for i in range(N_WARMUP):
                nc.tensor.matmul(
                    pw[:, :N_MM], lhsT=w_t[:, :H], rhs=w_t[:, :],
                    start=True, stop=True,
                )

            # Per-channel Toeplitz tables on the ACT ring (idle until stores
            # begin); separate tiles keep the dependency per channel.
            t_tiles = []
            for ch in range(CH_PER_CORE):
                k = KS[ch]
                t_c = tpool.tile([H, k * H], bf16, tag=f"t{ch}", name=f"t{ch}")
                nc.scalar.dma_start(
                    t_c[:, :], t_d[:, TOFF[ch] * H : (TOFF[ch] + k) * H]
                )
                t_tiles.append(t_c)

            for pair in range(N_PAIRS):
                x_t = xpool.tile([H, 2 * XCOLS], bf16, tag="x", name=f"x{pair}")
                if pair == 0:
                    # finest split so ch0's matmuls start as early as possible
                    nc.sync.dma_start(
                        x_t[:, :XSPLIT],
                        xp_d[0][:, :XSPLIT],
                    )
                    nc.sync.dma_start(
                        x_t[:, XSPLIT:XCOLS],
                        xp_d[0][:, XSPLIT:XCOLS],
                    )
                    nc.sync.dma_start(
                        x_t[:, XCOLS:],
                        xp_d[0][:, XCOLS:],
                    )
                elif pair == 1:
                    for c in range(2):
                        nc.sync.dma_start(
                            x_t[:, c * XCOLS : (c + 1) * XCOLS],
                            xp_d[1][:, c * XCOLS : (c + 1) * XCOLS],
                        )
                else:
                    nc.sync.dma_start(x_t[:, :], xp_d[pair])
                for c in range(2):
                    ch = 2 * pair + c
                    k = KS[ch]
                    pad = (k - 1) // 2
                    t_c = t_tiles[ch]
                    out_t = opool.tile([H, OCOLS], bf16, tag="o", name=f"o{ch}")
                    for half in range(N_BANKS // 2):
                        pt = pspool.tile(
                            [H, 2 * PS_STRIDE], f32, tag="ps", name=f"ps{ch}_{half}"
                        )
                        for u in range(2):
                            b = 2 * half + u
                            base = c * XCOLS + IMG_PER_BANK * b * RW
                            for dx in range(k):
                                off = dx - pad + DATA_OFF
                                nc.tensor.matmul(
                                    pt[:, u * PS_STRIDE : u * PS_STRIDE + N_MM],
                                    lhsT=t_c[:, dx * H : (dx + 1) * H],
                                    rhs=x_t[
                                        :, base + off : base + off + IMG_PER_BANK * RW
                                    ].rearrange("p (i r) -> p i r", i=IMG_PER_BANK)[
                                        :, :, :W
                                    ],
                                    start=(dx == 0),
                                    stop=(dx == k - 1),
                                )
                        nc.any.tensor_copy(
                            out=out_t.rearrange(
                                "p (g u i w) -> p g u i w",
                                g=N_BANKS // 2, u=2, i=IMG_PER_BANK,
                            )[:, half],
                            in_=pt.rearrange("p (u q) -> p u q", u=2)[:, :, :N_MM]
                            .rearrange("p u (i w) -> p u i w", i=IMG_PER_BANK),
                        )
                    if ch == CH_PER_CORE - 1:
                        # split the final store so the drain isn't gated on one
                        # full-channel transfer
                        nc.scalar.dma_start(
                            y_d[ch][:, : OCOLS // 2], out_t[:, : OCOLS // 2]
                        )
                        nc.scalar.dma_start(
                            y_d[ch][:, OCOLS // 2 :], out_t[:, OCOLS // 2 :]
                        )
                    else:
                        nc.scalar.dma_start(y_d[ch], out_t[:, :])
    nc.compile()
    return nc


def _get_bass():
    if "nc" not in _BASS_CACHE:
        _BASS_CACHE["nc"] = _build_bass()
    return _BASS_CACHE["nc"]


def _build_toeplitz(w, k):
    """w: [C, 1, k, k] -> T: [C, k, H, H], T[c,dx,hin,hout] = w[c,0,hin-hout+pad,dx]."""
    pad = (k - 1) // 2
    C = w.shape[0]
    T = np.zeros((C, k, H, H), np.float32)
    for dy in range(k):
        off = pad - dy  # hout = hin + off
        hin = np.arange(max(0, -off), H - max(0, off))
        T[:, :, hin, hin + off] = w[:, 0, dy, :][:, :, None]
    return T


def _core_channels(core):
    """Global channel ids for this core, in device (k-descending) order."""
    out = []
    for kg in GROUP_KS:
        gidx = {3: 0, 5: 1, 7: 2}[kg]
        base = gidx * GROUP_SIZE + core * CH_PER_GROUP_PER_CORE
        out.extend(range(base, base + CH_PER_GROUP_PER_CORE))
    return out


def _prepare_in_maps(x, w3, w5, w7):
    x = np.ascontiguousarray(np.asarray(x, dtype=np.float32))
    ws = {3: np.asarray(w3, np.float32), 5: np.asarray(w5, np.float32),
          7: np.asarray(w7, np.float32)}
    Ts = {k: _build_toeplitz(ws[k], k) for k in GROUP_KS}

    in_maps = []
    for core in range(N_CORES):
        chs = _core_channels(core)
        # staged x: [pair, H, (c, img, RW)] bf16, data at [3, 115) per region,
        # +6 zero slack cols at the end of each channel region
        xs = np.zeros((N_PAIRS, H, 2, N_IMGS, RW), ml_dtypes.bfloat16)
        xc = x[:, chs]  # [N, 24, H, W]
        xs[:, :, :, :, DATA_OFF : DATA_OFF + W] = (
            xc.transpose(1, 2, 0, 3)          # [24, H, N, W]
            .reshape(N_PAIRS, 2, H, N_IMGS, W)
            .transpose(0, 2, 1, 3, 4)         # [12, H, 2, N, W]
        )
        xs = xs.reshape(N_PAIRS, H, 2, N_IMGS * RW)
        xp = np.zeros((N_PAIRS, H, 2, XCOLS), ml_dtypes.bfloat16)
        xp[:, :, :, : N_IMGS * RW] = xs
        xp = np.ascontiguousarray(xp.reshape(N_PAIRS, H, 2 * XCOLS))

        # resident Toeplitz: [hin, (ch, dx, hout)] bf16, device channel order
        tm = np.concatenate(
            [
                Ts[kg][
                    core * CH_PER_GROUP_PER_CORE : (core + 1) * CH_PER_GROUP_PER_CORE
                ].reshape(-1, H, H)
                for kg in GROUP_KS
            ],
            axis=0,
        )  # [120, hin, hout]
        assert tm.shape[0] == N_TMAT
        tmd = np.ascontiguousarray(
            tm.transpose(1, 0, 2).reshape(H, TCOLS)
        ).astype(ml_dtypes.bfloat16)
        in_maps.append({"xp": xp, "tmat": tmd})
    return in_maps


def _gather(results):
    out = np.empty((N_IMGS, GROUP_SIZE * len(GROUP_KS), H, W), np.float32)
    for core in range(N_CORES):
        chs = _core_channels(core)
        y = results[core]["y"].astype(np.float32).reshape(CH_PER_CORE, H, N_IMGS, W)
        out[:, chs] = y.transpose(2, 0, 1, 3)
    return out


def run(x, w3, w5, w7, **spmd_kwargs):
    """Full run; returns (output, BassKernelResults) for profiling access."""
    nc = _get_bass()
    in_maps = _prepare_in_maps(x, w3, w5, w7)
    br = run_bass_kernel_spmd(nc, in_maps, core_ids=list(range(N_CORES)), **spmd_kwargs)
    return _gather(br.results), br


def kernel(x, w3, w5, w7):
    out, _ = run(x, w3, w5, w7)
    return out


# revision 12
# speedup vs baseline: 1.0143x; 1.0143x over previous
"""MixConv depthwise conv (3x3/5x5/7x7 over 64-channel groups) as banded-Toeplitz
matmuls on the TensorEngine, sharded over 8 NeuronCores by channel.

Decomposition: a kxk depthwise conv = sum over dx of a 1D conv along H applied to
the input shifted by dx along W. The 1D conv along H is a matmul with a banded
[H, H] Toeplitz matrix (built host-side from the conv weights) contracting over
H=112 partitions. W-shifts are free-dim offsets into a padded SBUF image tile;
the dx-passes accumulate in PSUM. Matmul rhs uses a segmented AP ([4 images x
112 cols], stride 118) so the inter-image pad columns are never streamed; PE
runs at the 1 col/cycle bf16 roofline with ~197 ns/matmul pitch.

Sharding: 192 channels / 8 cores = 24 channels per core, 8 from each kernel-size
group so PE work is balanced. Channels run k-descending (7,5,3) so DMA prefetch
builds headroom while the PE chews the big kernels.

All HBM traffic is bf16 (x, Toeplitz weights, y): fp32 staging makes the kernel
DMA-bound with the PE HAM-throttled. PSUM accumulation stays fp32; measured
~4e-3 max rel err vs the 2e-2 gate. Latency shaping: x loads ride the SP HWDGE
ring as channel-pair transfers (first pair split so ch0 starts early), y stores
and the per-channel Toeplitz tables ride the ACT ring, and a short burst of
dependency-free warmup matmuls on a memset scratch tile keeps the PE busy and
HAM-warm while the first transfers land.
"""

import numpy as np
import ml_dtypes

import concourse.bacc as bacc
import concourse.mybir as mybir
import concourse.tile as tile
from concourse.bass_utils import run_bass_kernel_spmd

# Problem constants (hardcoded per contract)
N_IMGS = 32
H = W = 112
GROUP_KS = (7, 5, 3)     # device processing order: k-descending
GROUP_SIZE = 64          # channels per group
N_CORES = 8
CH_PER_GROUP_PER_CORE = GROUP_SIZE // N_CORES   # 8
CH_PER_CORE = CH_PER_GROUP_PER_CORE * len(GROUP_KS)  # 24
N_PAIRS = CH_PER_CORE // 2

RW = W + 6               # per-image region width in the padded tile (max pad=3)
DATA_OFF = 3             # data cols at [3, 115) of each region
XCOLS = N_IMGS * RW + 6  # 3782 — +6 so the last bank's 4*RW slice stays in range
OCOLS = N_IMGS * W       # 3584
N_BANKS = 8              # image chunks / PSUM banks per channel
IMG_PER_BANK = N_IMGS // N_BANKS  # 4
N_MM = IMG_PER_BANK * W  # 448 — matmul free size (segmented, pads skipped)
PS_STRIDE = 512          # fp32 slots per PSUM bank; 2 banks per PSUM tile
N_WARMUP = 12            # dep-free matmuls to keep PE busy+warm during head DMAs
XSPLIT = RW * 16 + 8     # 1896 — ch0 half-load split (covers banks 0-3 reads)

KS = [7] * 8 + [5] * 8 + [3] * 8          # per-channel kernel size (device order)
TOFF = np.cumsum([0] + KS).tolist()       # tmat col-block offset per channel
N_TMAT = TOFF[-1]                          # 120 [H,H] Toeplitz slices
TCOLS = N_TMAT * H                         # 13440

MM_MODE = "bf16"

_BASS_CACHE = {}


def _build_bass():
    bf16 = mybir.dt.bfloat16
    f32 = mybir.dt.float32

    nc = bacc.Bacc("TRN2", target_bir_lowering=False, debug=False)
    xp_d = nc.dram_tensor(
        "xp", [N_PAIRS, H, 2 * XCOLS], bf16, kind="ExternalInput"
    )
    t_d = nc.dram_tensor("tmat", [H, TCOLS], bf16, kind="ExternalInput")
    y_d = nc.dram_tensor(
        "y", [CH_PER_CORE, H, OCOLS], bf16, kind="ExternalOutput"
    )

    with tile.TileContext(nc) as tc:
        with (
            tc.tile_pool(name="xpool", bufs=3) as xpool,
            tc.tile_pool(name="tpool", bufs=1) as tpool,
            tc.tile_pool(name="opool", bufs=3) as opool,
            tc.tile_pool(name="wpool", bufs=1) as wpool,
            tc.tile_pool(name="pspool", bufs=N_BANKS // 2, space="PSUM") as pspool,
        ):
            # PE warmup: dep-free matmuls on a memset tile fill the initial
            # DMA wait and take the HAM clock-gate to 8/8 before real work.
            w_t = wpool.tile([H, N_MM], bf16, tag="warm", name="warm")
            nc.any.memset(w_t[:, :], 0.0)
            pw = pspool.tile([H, 2 * PS_STRIDE], f32, tag="ps", name="ps_warm")
            for i in range(N_WARMUP):
                nc.tensor.matmul(
                    pw[:, :N_MM], lhsT=w_t[:, :H], rhs=w_t[:, :],
                    start=True, stop=True,
                )

            # Per-channel Toeplitz tables on the ACT ring (idle until stores
            # begin); separate tiles keep the dependency per channel. Only the
            # first two pairs' tables load upfront — the rest are prefetched
            # two pairs ahead so the head isn't DMA-aggregate-bound.
            t_tiles = [None] * CH_PER_CORE

            def load_tmat(ch):
                k = KS[ch]
                t_c = tpool.tile([H, k * H], bf16, tag=f"t{ch}", name=f"t{ch}")
                nc.scalar.dma_start(
                    t_c[:, :], t_d[:, TOFF[ch] * H : (TOFF[ch] + k) * H]
                )
                t_tiles[ch] = t_c

            for ch in range(4):
                load_tmat(ch)

            for pair in range(N_PAIRS):
                for c in range(2):
                    ch_pre = 2 * (pair + 2) + c
                    if ch_pre < CH_PER_CORE:
                        load_tmat(ch_pre)
                x_t = xpool.tile([H, 2 * XCOLS], bf16, tag="x", name=f"x{pair}")
                if pair == 0:
                    # finest split so ch0's matmuls start as early as possible
                    nc.sync.dma_start(
                        x_t[:, :XSPLIT],
                        xp_d[0][:, :XSPLIT],
                    )
                    nc.sync.dma_start(
                        x_t[:, XSPLIT:XCOLS],
                        xp_d[0][:, XSPLIT:XCOLS],
                    )
                    nc.sync.dma_start(
                        x_t[:, XCOLS:],
                        xp_d[0][:, XCOLS:],
                    )
                elif pair == 1:
                    for c in range(2):
                        nc.sync.dma_start(
                            x_t[:, c * XCOLS : (c + 1) * XCOLS],
                            xp_d[1][:, c * XCOLS : (c + 1) * XCOLS],
                        )
                else:
                    nc.sync.dma_start(x_t[:, :], xp_d[pair])
                for c in range(2):
                    ch = 2 * pair + c
                    k = KS[ch]
                    pad = (k - 1) // 2
                    t_c = t_tiles[ch]
                    out_t = opool.tile([H, OCOLS], bf16, tag="o", name=f"o{ch}")
                    for half in range(N_BANKS // 2):
                        pt = pspool.tile(
                            [H, 2 * PS_STRIDE], f32, tag="ps", name=f"ps{ch}_{half}"
                        )
                        for u in range(2):
                            b = 2 * half + u
                            base = c * XCOLS + IMG_PER_BANK * b * RW
                            for dx in range(k):
                                off = dx - pad + DATA_OFF
                                nc.tensor.matmul(
                                    pt[:, u * PS_STRIDE : u * PS_STRIDE + N_MM],
                                    lhsT=t_c[:, dx * H : (dx + 1) * H],
                                    rhs=x_t[
                                        :, base + off : base + off + IMG_PER_BANK * RW
                                    ].rearrange("p (i r) -> p i r", i=IMG_PER_BANK)[
                                        :, :, :W
                                    ],
                                    start=(dx == 0),
                                    stop=(dx == k - 1),
                                )
                        nc.any.tensor_copy(
                            out=out_t.rearrange(
                                "p (g u i w) -> p g u i w",
                                g=N_BANKS // 2, u=2, i=IMG_PER_BANK,
                            )[:, half],
                            in_=pt.rearrange("p (u q) -> p u q", u=2)[:, :, :N_MM]
                            .rearrange("p u (i w) -> p u i w", i=IMG_PER_BANK),
                        )
                    if ch == CH_PER_CORE - 1:
                        # split the final store so the drain isn't gated on one
                        # full-channel transfer
                        nc.scalar.dma_start(
                            y_d[ch][:, : OCOLS // 2], out_t[:, : OCOLS // 2]
                        )
                        nc.scalar.dma_start(
                            y_d[ch][:, OCOLS // 2 :], out_t[:, OCOLS // 2 :]
                        )
                    else:
                        nc.scalar.dma_start(y_d[ch], out_t[:, :])
    nc.compile()
    return nc


def _get_bass():
    if "nc" not in _BASS_CACHE:
        _BASS_CACHE["nc"] = _build_bass()
    return _BASS_CACHE["nc"]


def _build_toeplitz(w, k):
    """w: [C, 1, k, k] -> T: [C, k, H, H], T[c,dx,hin,hout] = w[c,0,hin-hout+pad,dx]."""
    pad = (k - 1) // 2
    C = w.shape[0]
    T = np.zeros((C, k, H, H), np.float32)
    for dy in range(k):
        off = pad - dy  # hout = hin + off
        hin = np.arange(max(0, -off), H - max(0, off))
        T[:, :, hin, hin + off] = w[:, 0, dy, :][:, :, None]
    return T


def _core_channels(core):
    """Global channel ids for this core, in device (k-descending) order."""
    out = []
    for kg in GROUP_KS:
        gidx = {3: 0, 5: 1, 7: 2}[kg]
        base = gidx * GROUP_SIZE + core * CH_PER_GROUP_PER_CORE
        out.extend(range(base, base + CH_PER_GROUP_PER_CORE))
    return out


def _prepare_in_maps(x, w3, w5, w7):
    x = np.ascontiguousarray(np.asarray(x, dtype=np.float32))
    ws = {3: np.asarray(w3, np.float32), 5: np.asarray(w5, np.float32),
          7: np.asarray(w7, np.float32)}
    Ts = {k: _build_toeplitz(ws[k], k) for k in GROUP_KS}

    in_maps = []
    for core in range(N_CORES):
        chs = _core_channels(core)
        # staged x: [pair, H, (c, img, RW)] bf16, data at [3, 115) per region,
        # +6 zero slack cols at the end of each channel region
        xs = np.zeros((N_PAIRS, H, 2, N_IMGS, RW), ml_dtypes.bfloat16)
        xc = x[:, chs]  # [N, 24, H, W]
        xs[:, :, :, :, DATA_OFF : DATA_OFF + W] = (
            xc.transpose(1, 2, 0, 3)          # [24, H, N, W]
            .reshape(N_PAIRS, 2, H, N_IMGS, W)
            .transpose(0, 2, 1, 3, 4)         # [12, H, 2, N, W]
        )
        xs = xs.reshape(N_PAIRS, H, 2, N_IMGS * RW)
        xp = np.zeros((N_PAIRS, H, 2, XCOLS), ml_dtypes.bfloat16)
        xp[:, :, :, : N_IMGS * RW] = xs
        xp = np.ascontiguousarray(xp.reshape(N_PAIRS, H, 2 * XCOLS))

        # resident Toeplitz: [hin, (ch, dx, hout)] bf16, device channel order
        tm = np.concatenate(
            [
                Ts[kg][
                    core * CH_PER_GROUP_PER_CORE : (core + 1) * CH_PER_GROUP_PER_CORE
                ].reshape(-1, H, H)
                for kg in GROUP_KS
            ],
            axis=0,
        )  # [120, hin, hout]
        assert tm.shape[0] == N_TMAT
        tmd = np.ascontiguousarray(
            tm.transpose(1, 0, 2).reshape(H, TCOLS)
        ).astype(ml_dtypes.bfloat16)
        in_maps.append({"xp": xp, "tmat": tmd})
    return in_maps


def _gather(results):
    out = np.empty((N_IMGS, GROUP_SIZE * len(GROUP_KS), H, W), np.float32)
    for core in range(N_CORES):
        chs = _core_channels(core)
        y = results[core]["y"].astype(np.float32).reshape(CH_PER_CORE, H, N_IMGS, W)
        out[:, chs] = y.transpose(2, 0, 1, 3)
    return out


def run(x, w3, w5, w7, **spmd_kwargs):
    """Full run; returns (output, BassKernelResults) for profiling access."""
    nc = _get_bass()
    in_maps = _prepare_in_maps(x, w3, w5, w7)
    br = run_bass_kernel_spmd(nc, in_maps, core_ids=list(range(N_CORES)), **spmd_kwargs)
    return _gather(br.results), br


def kernel(x, w3, w5, w7):
    out, _ = run(x, w3, w5, w7)
    return out


# revision 16
# speedup vs baseline: 1.0382x; 1.0236x over previous
"""MixConv depthwise conv (3x3/5x5/7x7 over 64-channel groups) as banded-Toeplitz
matmuls on the TensorEngine, sharded over 8 NeuronCores by channel.

Decomposition: a kxk depthwise conv = sum over dx of a 1D conv along H applied to
the input shifted by dx along W. The 1D conv along H is a matmul with a banded
[H, H] Toeplitz matrix (built host-side from the conv weights) contracting over
H=112 partitions. W-shifts are free-dim offsets into a padded SBUF image tile;
the dx-passes accumulate in PSUM. Matmul rhs uses a segmented AP ([4 images x
112 cols], stride 118) so the inter-image pad columns are never streamed; PE
runs at the 1 col/cycle bf16 roofline with ~197 ns/matmul pitch.

Sharding: 192 channels / 8 cores = 24 channels per core, 8 from each kernel-size
group so PE work is balanced. Channels run k-descending (7,5,3) so DMA prefetch
builds headroom while the PE chews the big kernels.

All HBM traffic is bf16 (x, Toeplitz weights, y): fp32 staging makes the kernel
DMA-bound with the PE HAM-throttled. PSUM accumulation stays fp32; measured
~4e-3 max rel err vs the 2e-2 gate. Latency shaping: x loads ride the SP HWDGE
ring as channel-pair transfers (first pair split so ch0 starts early), y stores
and the per-channel Toeplitz tables ride the ACT ring, and a short burst of
dependency-free warmup matmuls on a memset scratch tile keeps the PE busy and
HAM-warm while the first transfers land.
"""

import numpy as np
import ml_dtypes

import concourse.bacc as bacc
import concourse.mybir as mybir
import concourse.tile as tile
from concourse.bass_utils import run_bass_kernel_spmd

# Problem constants (hardcoded per contract)
N_IMGS = 32
H = W = 112
GROUP_KS = (7, 5, 3)     # device processing order: k-descending
GROUP_SIZE = 64          # channels per group
N_CORES = 8
CH_PER_GROUP_PER_CORE = GROUP_SIZE // N_CORES   # 8
CH_PER_CORE = CH_PER_GROUP_PER_CORE * len(GROUP_KS)  # 24
N_PAIRS = CH_PER_CORE // 2

RW = W + 6               # per-image region width in the padded tile (max pad=3)
DATA_OFF = 3             # data cols at [3, 115) of each region
XCOLS = N_IMGS * RW + 6  # 3782 — +6 so the last bank's 4*RW slice stays in range
OCOLS = N_IMGS * W       # 3584
N_BANKS = 8              # image chunks / PSUM banks per channel
IMG_PER_BANK = N_IMGS // N_BANKS  # 4
N_MM = IMG_PER_BANK * W  # 448 — matmul free size (segmented, pads skipped)
PS_STRIDE = 512          # fp32 slots per PSUM bank; 2 banks per PSUM tile
N_WARMUP = 12            # dep-free matmuls to keep PE busy+warm during head DMAs
XSPLIT = RW * 16 + 8     # 1896 — ch0 half-load split (covers banks 0-3 reads)

KS = [7] * 8 + [5] * 8 + [3] * 8          # per-channel kernel size (device order)
TOFF = np.cumsum([0] + KS).tolist()       # tmat col-block offset per channel
N_TMAT = TOFF[-1]                          # 120 [H,H] Toeplitz slices
TCOLS = N_TMAT * H                         # 13440

MM_MODE = "bf16"

_BASS_CACHE = {}


def _build_bass():
    bf16 = mybir.dt.bfloat16
    f32 = mybir.dt.float32

    nc = bacc.Bacc("TRN2", target_bir_lowering=False, debug=False)
    # per-pair row: [ch0 x | ch1 x | ch0 tmat | ch1 tmat] — the Toeplitz
    # tables ride the same DMA as their x data (same 112-partition layout)
    PCOLS = [2 * XCOLS + 2 * KS[2 * p] * H for p in range(N_PAIRS)]
    xp_ds = [
        nc.dram_tensor(f"xp{p}", [H, PCOLS[p]], bf16, kind="ExternalInput")
        for p in range(N_PAIRS)
    ]
    y_d = nc.dram_tensor(
        "y", [CH_PER_CORE, H, OCOLS], bf16, kind="ExternalOutput"
    )

    with tile.TileContext(nc) as tc:
        with (
            tc.tile_pool(name="xpool", bufs=3) as xpool,
            tc.tile_pool(name="opool", bufs=3) as opool,
            tc.tile_pool(name="wpool", bufs=1) as wpool,
            tc.tile_pool(name="pspool", bufs=N_BANKS // 2, space="PSUM") as pspool,
        ):
            # PE warmup: dep-free matmuls on a memset tile fill the initial
            # DMA wait and take the HAM clock-gate to 8/8 before real work.
            w_t = wpool.tile([H, N_MM], bf16, tag="warm", name="warm")
            nc.any.memset(w_t[:, :], 0.0)
            pw = pspool.tile([H, 2 * PS_STRIDE], f32, tag="ps", name="ps_warm")
            for i in range(N_WARMUP):
                nc.tensor.matmul(
                    pw[:, :N_MM], lhsT=w_t[:, :H], rhs=w_t[:, :],
                    start=True, stop=True,
                )

            for pair in range(N_PAIRS):
                pc = PCOLS[pair]
                x_t = xpool.tile([H, max(PCOLS)], bf16, tag="x", name=f"x{pair}")
                if pair <= 1:
                    # finest splits so the first channels' matmuls start early:
                    # tmat region first, then x in chunks
                    for a, b2 in [
                        (2 * XCOLS, pc),
                        (0, XSPLIT),
                        (XSPLIT, XCOLS),
                        (XCOLS, 2 * XCOLS),
                    ]:
                        nc.sync.dma_start(x_t[:, a:b2], xp_ds[pair][:, a:b2])
                else:
                    nc.sync.dma_start(x_t[:, :pc], xp_ds[pair][:, :])
                for c in range(2):
                    ch = 2 * pair + c
                    k = KS[ch]
                    pad = (k - 1) // 2
                    t_base = 2 * XCOLS + c * k * H
                    out_t = opool.tile([H, OCOLS], bf16, tag="o", name=f"o{ch}")
                    for half in range(N_BANKS // 2):
                        pt = pspool.tile(
                            [H, 2 * PS_STRIDE], f32, tag="ps", name=f"ps{ch}_{half}"
                        )
                        for u in range(2):
                            b = 2 * half + u
                            base = c * XCOLS + IMG_PER_BANK * b * RW
                            for dx in range(k):
                                off = dx - pad + DATA_OFF
                                nc.tensor.matmul(
                                    pt[:, u * PS_STRIDE : u * PS_STRIDE + N_MM],
                                    lhsT=x_t[
                                        :, t_base + dx * H : t_base + (dx + 1) * H
                                    ],
                                    rhs=x_t[
                                        :, base + off : base + off + IMG_PER_BANK * RW
                                    ].rearrange("p (i r) -> p i r", i=IMG_PER_BANK)[
                                        :, :, :W
                                    ],
                                    start=(dx == 0),
                                    stop=(dx == k - 1),
                                )
                        nc.any.tensor_copy(
                            out=out_t.rearrange(
                                "p (g u i w) -> p g u i w",
                                g=N_BANKS // 2, u=2, i=IMG_PER_BANK,
                            )[:, half],
                            in_=pt.rearrange("p (u q) -> p u q", u=2)[:, :, :N_MM]
                            .rearrange("p u (i w) -> p u i w", i=IMG_PER_BANK),
                        )
                    if ch == CH_PER_CORE - 1:
                        # split the final store so the drain isn't gated on one
                        # full-channel transfer
                        nc.scalar.dma_start(
                            y_d[ch][:, : OCOLS // 2], out_t[:, : OCOLS // 2]
                        )
                        nc.scalar.dma_start(
                            y_d[ch][:, OCOLS // 2 :], out_t[:, OCOLS // 2 :]
                        )
                    else:
                        nc.scalar.dma_start(y_d[ch], out_t[:, :])
    nc.compile()
    return nc


def _get_bass():
    if "nc" not in _BASS_CACHE:
        _BASS_CACHE["nc"] = _build_bass()
    return _BASS_CACHE["nc"]


def _build_toeplitz(w, k):
    """w: [C, 1, k, k] -> T: [C, k, H, H], T[c,dx,hin,hout] = w[c,0,hin-hout+pad,dx]."""
    pad = (k - 1) // 2
    C = w.shape[0]
    T = np.zeros((C, k, H, H), np.float32)
    for dy in range(k):
        off = pad - dy  # hout = hin + off
        hin = np.arange(max(0, -off), H - max(0, off))
        T[:, :, hin, hin + off] = w[:, 0, dy, :][:, :, None]
    return T


def _core_channels(core):
    """Global channel ids for this core, in device (k-descending) order."""
    out = []
    for kg in GROUP_KS:
        gidx = {3: 0, 5: 1, 7: 2}[kg]
        base = gidx * GROUP_SIZE + core * CH_PER_GROUP_PER_CORE
        out.extend(range(base, base + CH_PER_GROUP_PER_CORE))
    return out


def _prepare_in_maps(x, w3, w5, w7):
    x = np.ascontiguousarray(np.asarray(x, dtype=np.float32))
    ws = {3: np.asarray(w3, np.float32), 5: np.asarray(w5, np.float32),
          7: np.asarray(w7, np.float32)}
    Ts = {k: _build_toeplitz(ws[k], k) for k in GROUP_KS}

    in_maps = []
    for core in range(N_CORES):
        chs = _core_channels(core)
        # staged x: [pair, H, (c, img, RW)] bf16, data at [3, 115) per region,
        # +6 zero slack cols at the end of each channel region
        xs = np.zeros((N_PAIRS, H, 2, N_IMGS, RW), ml_dtypes.bfloat16)
        xc = x[:, chs]  # [N, 24, H, W]
        xs[:, :, :, :, DATA_OFF : DATA_OFF + W] = (
            xc.transpose(1, 2, 0, 3)          # [24, H, N, W]
            .reshape(N_PAIRS, 2, H, N_IMGS, W)
            .transpose(0, 2, 1, 3, 4)         # [12, H, 2, N, W]
        )
        xs = xs.reshape(N_PAIRS, H, 2, N_IMGS * RW)
        xp = np.zeros((N_PAIRS, H, 2, XCOLS), ml_dtypes.bfloat16)
        xp[:, :, :, : N_IMGS * RW] = xs

        # per-channel Toeplitz [hin, (dx, hout)] bf16, appended to the pair row
        tm = np.concatenate(
            [
                Ts[kg][
                    core * CH_PER_GROUP_PER_CORE : (core + 1) * CH_PER_GROUP_PER_CORE
                ].reshape(-1, H, H)
                for kg in GROUP_KS
            ],
            axis=0,
        )  # [120, hin, hout]
        assert tm.shape[0] == N_TMAT
        tmd = tm.transpose(1, 0, 2).astype(ml_dtypes.bfloat16)  # [hin, 120, hout]

        in_map = {}
        for p in range(N_PAIRS):
            k = KS[2 * p]
            trow = tmd[:, TOFF[2 * p] : TOFF[2 * p + 2], :].reshape(H, 2 * k * H)
            in_map[f"xp{p}"] = np.ascontiguousarray(
                np.concatenate(
                    [xp[p].reshape(H, 2 * XCOLS), trow], axis=1
                )
            )
        in_maps.append(in_map)
    return in_maps


def _gather(results):
    out = np.empty((N_IMGS, GROUP_SIZE * len(GROUP_KS), H, W), np.float32)
    for core in range(N_CORES):
        chs = _core_channels(core)
        y = results[core]["y"].astype(np.float32).reshape(CH_PER_CORE, H, N_IMGS, W)
        out[:, chs] = y.transpose(2, 0, 1, 3)
    return out


def run(x, w3, w5, w7, **spmd_kwargs):
    """Full run; returns (output, BassKernelResults) for profiling access."""
    nc = _get_bass()
    in_maps = _prepare_in_maps(x, w3, w5, w7)
    br = run_bass_kernel_spmd(nc, in_maps, core_ids=list(range(N_CORES)), **spmd_kwargs)
    return _gather(br.results), br


def kernel(x, w3, w5, w7):
    out, _ = run(x, w3, w5, w7)
    return out
